# revision 20
# baseline (speedup 1.0000x reference)
"""Trainium2 Bass kernel for nn_DocREModel (8-core SPMD), v2.

Sharding: 4 docs x 2 half-bands = 8 cores. Each core runs an identical
program. The pair of cores sharing a doc splits the conv stack spatially:
each computes conv3 output rows 0..10 in ITS OWN entity coordinates.
Odd cores see the document with entity order reflected (e -> 21-e) and
spatially-flipped conv kernels, which makes "rows 0..10 local" equal to
global rows 11..21 — so both halves run the same compiled program.

Host does only index-driven data movement: batch slicing, transposes,
row gathers at integer indices, one-hot/selector construction, dtype
casts, and packing into wide-row DMA blobs. All FP arithmetic runs on
device.
"""

import numpy as np
from contextlib import ExitStack

import concourse.bass as bass
import concourse.bacc as bacc
import concourse.tile as tile
import concourse.mybir as mybir
from concourse.bass_utils import run_bass_kernel_spmd

import ml_dtypes

FP32 = mybir.dt.float32
BF16 = mybir.dt.bfloat16
FP8 = mybir.dt.float8e4
_NPDT = {FP32: np.float32, BF16: ml_dtypes.bfloat16}

S_ATT = 128.0     # attm fp8 scale (cancels in ea row-normalization)
S_X = 16.0        # xn fp8 scale (folded into rcp)

B, C, H, NH = 4, 1024, 768, 12
E, M, L, LS = 22, 3, 30, 16
NN, NF, EMB = 118, 532, 512
P, PH2, IC = 462, 256, 256
S = 22
ACT = mybir.ActivationFunctionType

# conv split geometry (local coords; identical on every core)
NX, N1, N2, N3 = 17, 15, 13, 11      # x rows 0..16 -> c1 0..14 -> c2 0..12 -> c3 0..10
PW = 26                               # padded cols (-2..23)
SP3 = N3 * S                          # 242 spatial positions of x3
SP3_TILES = [(0, 128), (128, SP3 - 128)]   # 128 + 114

ATTM_TILES = 7                        # 792 rows -> 7x128 (pad)
SPAN_TILES = 4                        # 480 rows -> 4x128 (pad)
GM_TILES = 7

# ---- C0..C4 blob16 layouts: (name, rows, cols[, count]) ----
def _layout(entries):
    off, d = 0, {}
    for name, cols in entries:
        d[name] = off
        off += cols
    return d, off

NODE_GROUPS = [22, 66, 30]            # entities, mentions, links

C0A_L, C0A_W = _layout([
    ("adjrel0", 110), ("adjrel1", 110), ("adjrel2", 110),
    ("typ0", 20), ("typ1", 20), ("typ2", 20),
    ("g3", 22), ("ones16", 1), ("gmat", GM_TILES * 22),
    ("gspan", SPAN_TILES * 30), ("attl", SPAN_TILES * 192),
])
C0B_L, C0B_W = _layout([("xg", 6 * 66), ("wtrans", 6 * 512)])
C4_L, C4_W = _layout([("sh", PH2), ("st", PH2), ("sm", 2 * PH2)])
C1_W = 4 * 768        # xspan
C2_W = 7 * 1024       # attm
C3_W = 8 * 768        # xn

WCH = 13 * 512            # unified weight-chunk width (6656 cols bf16)

# blob32 layout (full 128-partition fields only)
B32_L, B32_W = _layout([
    ("ident", 128), ("ones", 1),
    ("b1", 2), ("b2", 2), ("b3", 4), ("bht", 8), ("bbil", 1),
])
# single-partition row blob: [1, BROW_W]
BROW_L, BROW_W = _layout([("onesrow", 128), ("btrans", 512), ("brgcn", 512)])


def build_program():
    nc = bacc.Bacc("TRN2", target_bir_lowering=False, debug=False)

    def din(name, shape, dt=BF16):
        return nc.dram_tensor(name, shape, dt, kind="ExternalInput").ap()

    identp_d = din("identp", [128, 128])
    c0a_d = din("c0a", [128, C0A_W])
    c0b_d = din("c0b", [128, C0B_W])
    c1_d = din("c1", [128, C1_W])
    c2_d = din("c2", [128, C2_W], FP8)
    c3_d = din("c3", [128, C3_W], FP8)
    c4_d = din("c4", [128, C4_W])
    b32_d = din("b32", [128, B32_W], FP32)
    brow_d = din("brow", [1, BROW_W], FP32)
    wrel_d = din("wrel", [128, 20 * 512])   # 20 full blocks (r, kt<4)
    wtail_d = din("wtail", [20, 5 * 512])   # 5 tail blocks (kt=4, 20 rows)
    w1_d = din("w1", [128, 4 * WCH])
    w2_d = din("w2", [128, 2 * WCH])
    w3_d = din("w3", [128, 4 * WCH])
    wht_d = din("wht", [128, 16 * 1024])
    wbil_d = din("wbil", [128, 8 * 97])
    outt = nc.dram_tensor("outt", [97, PH2], FP32, kind="ExternalOutput").ap()

    with tile.TileContext(nc) as tc, ExitStack() as ctx:
        pp = ctx.enter_context(tc.tile_pool(name="persist", bufs=1))
        pst = ctx.enter_context(tc.tile_pool(name="stream", bufs=1))
        pps = ctx.enter_context(tc.tile_pool(name="psum", bufs=4, space="PSUM"))
        pdram = ctx.enter_context(tc.tile_pool(name="dram", bufs=1, space="DRAM"))

        dma = nc.sync.dma_start
        gdma = nc.gpsimd.dma_start

        def T(pool, shape, dt, tag, bufs=None):
            return pool.tile(shape, dt, tag=tag, name=tag, bufs=bufs)

        # ---- DMAs (order within each ring matters) ----
        # ring order = priority: graph-phase inputs first on BOTH rings,
        # weight streams after (weight-ring slots gate the rest).
        identp_t = T(pp, [128, 128], BF16, "identp")
        dma(identp_t[:], identp_d)
        # gpsimd ring: c1, c3, wrel, wtail, w1k2, w1k3, w2
        c1_t = T(pp, [128, C1_W], BF16, "c1")
        gdma(c1_t[:], c1_d)
        c3_t = T(pp, [128, C3_W], FP8, "c3")
        gdma(c3_t[:], c3_d)
        # sync ring: identp, b32, brow, c0, c2, c4, w1k0, w1k1, wht, w3, wbil
        b32_t = T(pp, [128, B32_W], FP32, "b32")
        dma(b32_t[:], b32_d)
        brow_t = T(pp, [1, BROW_W], FP32, "brow")
        dma(brow_t[:], brow_d)
        c0b_t = T(pp, [128, C0B_W], BF16, "c0b")
        dma(c0b_t[:], c0b_d)
        c0a_t = T(pp, [128, C0A_W], BF16, "c0a")
        dma(c0a_t[:], c0a_d)
        # unified weight-chunk ring: wrel(2), w1(4), w2(2), w3(4)
        def wchunk(dram, k, engine_dma, used=WCH):
            t = T(pst, [128, WCH], BF16, "wchunk", bufs=4)
            engine_dma(t[:, 0:used], dram[:, k * WCH:k * WCH + used])
            return t

        wrel_t = [wchunk(wrel_d, 0, dma),
                  wchunk(wrel_d, 1, dma, used=7 * 512)]
        wtail_t = T(pp, [20, 5 * 512], BF16, "wtail")
        dma(wtail_t[:], wtail_d)
        # attm fp8 (2 chunks)
        c2_t = []
        for k, used in enumerate([4096, C2_W - 4096]):
            t = T(pst, [128, 4096], FP8, "c2s", bufs=2)
            gdma(t[:, 0:used], c2_d[:, k * 4096:k * 4096 + used])
            c2_t.append(t)
        c4_t = T(pp, [128, C4_W], BF16, "c4")
        dma(c4_t[:], c4_d)
        w1_t = [wchunk(w1_d, 0, dma), wchunk(w1_d, 1, dma),
                wchunk(w1_d, 2, gdma), wchunk(w1_d, 3, gdma)]
        w2_t = [wchunk(w2_d, k, gdma) for k in range(2)]
        wht_t = []
        for k in range(4):
            t = T(pst, [128, 4096], BF16, "whts", bufs=4)
            dma(t[:], wht_d[:, k * 4096:(k + 1) * 4096])
            wht_t.append(t)
        w3_t = [wchunk(w3_d, k, dma) for k in range(4)]
        wbil_t = T(pp, [128, 8 * 97], BF16, "wbil")
        dma(wbil_t[:], wbil_d)

        def c0s(name, r0, r1, w):
            if name in C0A_L:
                return c0a_t[r0:r1, C0A_L[name]:C0A_L[name] + w]
            return c0b_t[r0:r1, C0B_L[name]:C0B_L[name] + w]

        ident = b32_t[:, B32_L["ident"]:B32_L["ident"] + 128]
        onesrow = brow_t[0:1, BROW_L["onesrow"]:BROW_L["onesrow"] + 128]

        # ---- keep-warm matmul batteries: hold PE HAM at 8/8 through the
        # DMA-gated graph phase (an idle MID window re-throttles to 1.2GHz)
        ps_warm = T(pps, [128, PH2], FP32, "psht", bufs=4)
        # memset-born operand: warmup starts before any DMA lands
        warm_src = T(pp, [128, 128], BF16, "warm_src")
        nc.vector.memset(warm_src[:], 0.001)

        def warm(n):
            for _ in range(n):
                nc.tensor.matmul(ps_warm[:, 0:128], warm_src[:], warm_src[:],
                                 start=True, stop=True)

        warm(48)

        # ---- broadcast rows via K=1 matmuls ----
        def bcast_row(src_row_ap, w, tag):
            ps = T(pps, [128, w], FP32, "ps")
            nc.tensor.matmul(ps[:], onesrow, src_row_ap, start=True, stop=True)
            t = T(pp, [128, w], FP32, tag)
            nc.vector.tensor_copy(t[:], ps[:])
            return t

        btrans_bc = bcast_row(brow_t[0:1, BROW_L["btrans"]:BROW_L["btrans"] + 512],
                              512, "btrans_bc")
        brgcn_bc = bcast_row(brow_t[0:1, BROW_L["brgcn"]:BROW_L["brgcn"] + 512],
                             512, "brgcn_bc")

        nodes_g = [T(pp, [g, NF], BF16, f"nodes{gi}")
                   for gi, g in enumerate(NODE_GROUPS)]
        for gi, g in enumerate(NODE_GROUPS):
            nc.vector.tensor_copy(nodes_g[gi][:, EMB:NF],
                                  c0s(f"typ{gi}", 0, g, 20))

        wtrans = [c0s("wtrans", 0, 128, 3072)[:, k * 512:(k + 1) * 512]
                  for k in range(6)]

        # ---- adjacency normalization (needs only c0a) ----
        ps_cs = T(pps, [1, 88], FP32, "ps")
        for gi, g in enumerate(NODE_GROUPS):
            nc.tensor.matmul(ps_cs[:], c0s("ones16", 0, g, 1),
                             c0s(f"adjrel{gi}", 0, g, 110)[:, 0:88],
                             start=(gi == 0), stop=(gi == 2))
        cs = T(pp, [1, 88], FP32, "cs")
        nc.vector.tensor_scalar_add(cs[:], ps_cs[:], 1e-5)
        csr = T(pp, [1, 88], FP32, "csr")
        nc.vector.reciprocal(csr[:], cs[:])
        ps_csb = T(pps, [128, 88], FP32, "ps")
        nc.tensor.matmul(ps_csb[:], onesrow, csr[:], start=True, stop=True)
        csb = T(pp, [128, 88], FP32, "csb")
        nc.vector.tensor_copy(csb[:], ps_csb[:])
        adjn_g = []
        for gi, g in enumerate(NODE_GROUPS):
            t = T(pp, [g, 110], BF16, f"adjn{gi}")
            nc.vector.tensor_mul(t[:, 0:88],
                                 c0s(f"adjrel{gi}", 0, g, 110)[:, 0:88],
                                 csb[0:g, :])
            nc.vector.tensor_copy(t[:, 88:110],
                                  c0s(f"adjrel{gi}", 0, g, 110)[:, 88:110])
            adjn_g.append(t)

        # ---- S2: mention embeddings + entity logsumexp ----
        warm(40)
        ps_memb = T(pps, [E * M, EMB], FP32, "ps")
        for kt in range(6):
            nc.tensor.matmul(ps_memb[:],
                             c0s("xg", 0, 128, 396)[:, kt * 66:(kt + 1) * 66],
                             wtrans[kt], start=(kt == 0), stop=(kt == 5))
        memb = T(pp, [E * M, EMB], FP32, "memb")
        nc.vector.tensor_add(memb[:], ps_memb[:], btrans_bc[0:E * M, :])
        nc.vector.tensor_copy(nodes_g[1][:, 0:EMB], memb[:])
        ememb = T(pp, [E * M, EMB], BF16, "ememb")
        nc.scalar.activation(ememb[:], memb[:], ACT.Exp)
        ps_ent = T(pps, [E, EMB], FP32, "ps")
        nc.tensor.matmul(ps_ent[:], c0s("g3", 0, E * M, 22), ememb[:],
                         start=True, stop=True)
        nc.scalar.activation(nodes_g[0][:, 0:EMB], ps_ent[:], ACT.Ln)

        # ---- S3: link nodes ----
        aT, aTb = [], []
        for i in range(SPAN_TILES):
            al = c0s("attl", 0, 128, SPAN_TILES * 192)[:, i * 192:(i + 1) * 192]
            a = T(pp, [128, 1], FP32, f"aT{i}")
            nc.vector.tensor_reduce(a[:], al, mybir.AxisListType.X,
                                    mybir.AluOpType.add)
            nc.vector.tensor_scalar_mul(a[:], a[:], 1.0 / 192.0)
            aT.append(a)
            ab = T(pp, [128, 1], BF16, f"aTb{i}")
            nc.vector.tensor_copy(ab[:], a[:])
            aTb.append(ab)
        gspan = [c0s("gspan", 0, 128, SPAN_TILES * 30)[:, i * 30:(i + 1) * 30]
                 for i in range(SPAN_TILES)]
        ps_as = T(pps, [L, 1], FP32, "ps")
        for i in range(SPAN_TILES):
            nc.tensor.matmul(ps_as[:], gspan[i], aTb[i][:],
                             start=(i == 0), stop=(i == SPAN_TILES - 1))
        asum = T(pp, [L, 1], FP32, "asum")
        nc.vector.tensor_copy(asum[:], ps_as[:])
        for i in range(SPAN_TILES):
            nc.vector.tensor_scalar_mul(
                c1_t[:, i * 768:(i + 1) * 768],
                c1_t[:, i * 768:(i + 1) * 768], aT[i][:])
        warm(12)
        lct = []
        for mt in range(6):
            ps = T(pps, [128, L], FP32, "ps")
            for i in range(SPAN_TILES):
                nc.tensor.matmul(
                    ps[:], c1_t[:, i * 768 + mt * 128:i * 768 + mt * 128 + 128],
                    gspan[i], start=(i == 0), stop=(i == SPAN_TILES - 1))
            t = T(pp, [128, L], BF16, f"lct{mt}")
            nc.vector.tensor_copy(t[:], ps[:])
            lct.append(t)
        bterm = T(pp, [L, EMB], FP32, "bterm")
        nc.vector.tensor_scalar_mul(bterm[:], btrans_bc[0:L, :], asum[:])
        ps_link = T(pps, [L, EMB], FP32, "ps")
        for kt in range(6):
            nc.tensor.matmul(ps_link[:], lct[kt][:], wtrans[kt],
                             start=(kt == 0), stop=(kt == 5))
        nc.vector.tensor_add(nodes_g[2][:, 0:EMB], ps_link[:], bterm[:])

        # ---- S5: RGCN (entity outputs only) — before S4 on the PE queue;
        # it depends on nodes+wrel, not on the attm/xn chunks.
        warm(16)
        NF_TILES = [(0, 128), (128, 128), (256, 128), (384, 128), (512, 20)]
        msgT = []
        for i, (off, sz) in enumerate(NF_TILES):
            ps = T(pps, [sz, 110], FP32, "ps")
            for gi in range(3):
                nc.tensor.matmul(ps[:], nodes_g[gi][:, off:off + sz],
                                 adjn_g[gi][:], start=(gi == 0), stop=(gi == 2))
            t = T(pp, [sz, 110], BF16, f"msgT{i}")
            nc.vector.tensor_copy(t[:], ps[:])
            msgT.append(t)
        ps_gcn = T(pps, [E, EMB], FP32, "ps")
        for r in range(5):
            for i, (off, sz) in enumerate(NF_TILES):
                if i < 4:
                    bidx = r * 4 + i
                    wv = wrel_t[bidx // 13][0:sz, (bidx % 13) * 512:
                                            (bidx % 13) * 512 + 512]
                else:
                    wv = wtail_t[0:20, r * 512:(r + 1) * 512]
                nc.tensor.matmul(
                    ps_gcn[:], msgT[i][:, r * 22:(r + 1) * 22], wv,
                    start=(r == 0 and i == 0), stop=(r == 4 and i == 4))
        ent = T(pp, [E, EMB], FP32, "ent")
        nc.vector.tensor_add(ent[:], ps_gcn[:], brgcn_bc[0:E, :])
        nc.scalar.activation(ent[:], ent[:], ACT.Relu)
        ent16 = T(pp, [E, EMB], BF16, "ent16")
        nc.vector.tensor_copy(ent16[:], ent[:])
        entT = []
        for mt in range(4):
            ps = T(pps, [128, E], FP32, "ps")
            nc.tensor.transpose(ps[:], ent[:, mt * 128:(mt + 1) * 128],
                                ident[0:E, 0:E])
            t = T(pp, [128, E], FP32, f"entT{mt}")
            nc.vector.tensor_copy(t[:], ps[:])
            entT.append(t)
        # precompute the ent-outer-product half of the x map now, so only
        # the ectx half remains on the critical path after S4
        xt1 = []
        for mt in range(4):
            t = T(pp, [128, NX * S], FP32, f"xt1_{mt}")
            nc.vector.tensor_mul(
                t[:].rearrange("p (a b) -> p a b", a=NX, b=S),
                entT[mt][:, 0:NX].unsqueeze(2).to_broadcast((128, NX, S)),
                entT[mt][:].unsqueeze(1).to_broadcast((128, NX, S)))
            xt1.append(t)

        # ---- S4: ea, then e_ctx = rcp * ((ea @ X) @ W) + b ----
        warm(12)
        ps_ea = [T(pps, [E, 512], FP32, "ps") for _ in range(2)]
        for i in range(ATTM_TILES):
            at = c2_t[i // 4][:, (i % 4) * 1024:(i % 4) * 1024 + 1024]
            gt = c0s("gmat", 0, 128, GM_TILES * 22)[:, i * 22:(i + 1) * 22]
            for half in range(2):
                nc.tensor.matmul(ps_ea[half][:], gt,
                                 at[:, half * 512:(half + 1) * 512],
                                 start=(i == 0), stop=(i == ATTM_TILES - 1))
        ea = T(pp, [E, C], FP32, "ea")
        for half in range(2):
            nc.vector.tensor_copy(ea[:, half * 512:(half + 1) * 512],
                                  ps_ea[half][:])
        rsum = T(pp, [E, 1], FP32, "rsum")
        nc.vector.tensor_reduce(rsum[:], ea[:], mybir.AxisListType.X,
                                mybir.AluOpType.add)
        # ea is scaled by S_ATT; eps scales with it, and 1/S_X folds into rcp
        nc.vector.tensor_scalar_add(rsum[:], rsum[:], S_ATT * 1e-5)
        rcp = T(pp, [E, 1], FP32, "rcp")
        nc.vector.reciprocal(rcp[:], rsum[:])
        nc.vector.tensor_scalar_mul(rcp[:], rcp[:], 1.0 / S_X)
        eaT = []
        for kt in range(8):
            ps = T(pps, [128, E], FP32, "ps")
            nc.tensor.transpose(ps[:], ea[:, kt * 128:(kt + 1) * 128],
                                ident[0:E, 0:E])
            t = T(pp, [128, E], BF16, f"eaT{kt}")
            nc.vector.tensor_copy(t[:], ps[:])
            eaT.append(t)
        warm(12)
        uT = []
        for mt in range(6):
            ps = T(pps, [128, E], FP32, "ps")
            for kt in range(8):
                nc.tensor.matmul(
                    ps[:], c3_t[:, kt * 768 + mt * 128:kt * 768 + mt * 128 + 128],
                    eaT[kt][:], start=(kt == 0), stop=(kt == 7))
            t = T(pp, [128, E], BF16, f"uT{mt}")
            nc.vector.tensor_copy(t[:], ps[:])
            uT.append(t)
        ps_ectx = T(pps, [E, EMB], FP32, "ps")
        for kt in range(6):
            nc.tensor.matmul(ps_ectx[:], uT[kt][:], wtrans[kt],
                             start=(kt == 0), stop=(kt == 5))
        ectx = T(pp, [E, EMB], FP32, "ectx")
        nc.vector.tensor_scalar_mul(ectx[:], ps_ectx[:], rcp[:])
        nc.vector.tensor_add(ectx[:], ectx[:], btrans_bc[0:E, :])
        ectxT = []
        for mt in range(4):
            ps = T(pps, [128, E], FP32, "ps")
            nc.tensor.transpose(ps[:], ectx[:, mt * 128:(mt + 1) * 128],
                                ident[0:E, 0:E])
            t = T(pp, [128, E], FP32, f"ectxT{mt}")
            nc.vector.tensor_copy(t[:], ps[:])
            ectxT.append(t)

        # ---- S6: x maps + conv stack (local rows only) ----
        xpad = []
        for mt in range(4):
            xp = T(pp, [128, (NX + 4) * PW], BF16, f"xpad{mt}")
            nc.vector.memset(xp[:], 0.0)
            t2 = T(pst, [128, NX * S], FP32, "xtmp", bufs=2)
            nc.vector.tensor_mul(
                t2[:].rearrange("p (a b) -> p a b", a=NX, b=S),
                ectxT[mt][:, 0:NX].unsqueeze(2).to_broadcast((128, NX, S)),
                ectxT[mt][:].unsqueeze(1).to_broadcast((128, NX, S)))
            inner = xp[:].rearrange("p (a b) -> p a b", a=NX + 4, b=PW)[
                :, 2:2 + NX, 2:2 + S]
            nc.vector.tensor_add(inner, xt1[mt][:], t2[:])
            xpad.append(xp)

        def conv(in_tiles, in_rows, wsel, oc, n_out, out_cb):
            """kt-outer 5x5 conv so weight chunks are consumed sequentially."""
            n_ic_t, n_oc_t = len(in_tiles), oc // 128
            ps_c = [T(pps, [128, n_out * S], FP32, "ps") for _ in range(n_oc_t)]
            n_acc = 25 * n_ic_t
            a = 0
            for kt in range(n_ic_t):
                for tap in range(25):
                    di, dj = divmod(tap, 5)
                    rhs = in_tiles[kt][:].rearrange(
                        "p (a b) -> p a b", a=in_rows + 4, b=PW)[
                        :, di:di + n_out, dj:dj + S]
                    w = wsel(kt, tap)
                    for mt in range(n_oc_t):
                        nc.tensor.matmul(
                            ps_c[mt][:], w[:, mt * 128:mt * 128 + 128],
                            rhs, start=(a == 0), stop=(a == n_acc - 1))
                    a += 1
            for mt in range(n_oc_t):
                out_cb(mt, ps_c[mt])

        pad1 = []
        for mt in range(2):
            t = T(pp, [128, (N1 + 4) * PW], BF16, f"pad1_{mt}")
            nc.vector.memset(t[:], 0.0)
            pad1.append(t)

        warm(8)
        warm_sb = T(pp, [128, 128], FP32, "warm_sb")
        nc.vector.tensor_copy(warm_sb[:], ps_warm[:, 0:128])
        warm_dram = pdram.tile([128, 128], FP32, name="warm_dram")
        dma(warm_dram[:], warm_sb[:])

        def c1_out(mt, ps):
            inner = pad1[mt][:].rearrange("p (a b) -> p a b",
                                          a=N1 + 4, b=PW)[:, 2:2 + N1, 2:2 + S]
            nc.scalar.activation(
                inner, ps[:].rearrange("p (a b) -> p a b", a=N1, b=S),
                ACT.Relu, bias=b32_t[:, B32_L["b1"] + mt:B32_L["b1"] + mt + 1])

        conv(xpad, NX, lambda kt, tap: w1_t[kt][:, tap * IC:tap * IC + IC],
             IC, N1, c1_out)

        pad2 = []
        for mt in range(2):
            t = T(pp, [128, (N2 + 4) * PW], BF16, f"pad2_{mt}")
            nc.vector.memset(t[:], 0.0)
            pad2.append(t)

        def c2_out(mt, ps):
            inner = pad2[mt][:].rearrange("p (a b) -> p a b",
                                          a=N2 + 4, b=PW)[:, 2:2 + N2, 2:2 + S]
            nc.scalar.activation(
                inner, ps[:].rearrange("p (a b) -> p a b", a=N2, b=S),
                ACT.Relu, bias=b32_t[:, B32_L["b2"] + mt:B32_L["b2"] + mt + 1])

        conv(pad1, N1, lambda kt, tap: w2_t[kt][:, tap * IC:tap * IC + IC],
             IC, N2, c2_out)

        # ---- S7a: pair features not needing conv3 — run inside conv window
        sh = c4_t[0:E, C4_L["sh"]:C4_L["sh"] + PH2]
        st = c4_t[0:E, C4_L["st"]:C4_L["st"] + PH2]
        sm = [c4_t[:, C4_L["sm"] + i * PH2:C4_L["sm"] + (i + 1) * PH2]
              for i in range(2)]
        featT = [None] * 16
        for j, sel_ap in ((0, sh), (4, st)):
            for mt in range(4):
                ps = T(pps, [128, PH2], FP32, "ps")
                nc.tensor.matmul(ps[:], ent16[:, mt * 128:(mt + 1) * 128],
                                 sel_ap, start=True, stop=True)
                t = T(pp, [128, PH2], BF16, f"featT{j + mt}")
                nc.vector.tensor_copy(t[:], ps[:])
                featT[j + mt] = t
        for mt in range(4):
            t = T(pp, [128, PH2], BF16, f"featT{12 + mt}")
            nc.vector.tensor_mul(t[:], featT[mt][:], featT[4 + mt][:])
            featT[12 + mt] = t
        ps_ht = [T(pps, [128, PH2], FP32, "psht", bufs=4) for _ in range(4)]

        def ht_block(ps_tiles, mts, kts, first, last):
            for kt in kts:
                w = wht_t[kt // 4]
                for j, mt in enumerate(mts):
                    nc.tensor.matmul(
                        ps_tiles[j][:],
                        w[:, (kt % 4) * 1024 + mt * 128:
                          (kt % 4) * 1024 + mt * 128 + 128],
                        featT[kt][:], start=(first and kt == kts[0]),
                        stop=(last and kt == kts[-1]))

        ht_block(ps_ht, [0, 1, 2, 3], list(range(8)) + [12, 13, 14, 15],
                 True, False)

        x3 = [T(pp, [128, SP3], BF16, f"x3_{mt}") for mt in range(4)]

        def c3_out(mt, ps):
            nc.scalar.activation(
                x3[mt][:], ps[:], ACT.Relu,
                bias=b32_t[:, B32_L["b3"] + mt:B32_L["b3"] + mt + 1])

        def w3sel(kt, tap):
            half = tap // 13
            lo = (tap - 13 * half) * EMB
            return w3_t[kt * 2 + half][:, lo:lo + EMB]

        conv(pad2, N2, w3sel, EMB, N3, c3_out)

        # ---- S7: pair features + classifier ----
        x3T = []
        for i, (off, sz) in enumerate(SP3_TILES):
            t = T(pp, [sz, EMB], BF16, f"x3T{i}")
            x3T.append(t)
            for src in range(4):
                ps = T(pps, [sz, 64], FP32, "ps")
                psb = ps[:].bitcast(BF16)
                nc.tensor.transpose(psb, x3[src][:, off:off + sz],
                                    identp_t[:, :])
                nc.vector.tensor_copy(t[:, src * 128:(src + 1) * 128], psb)

        for mt in range(4):
            ps = T(pps, [128, PH2], FP32, "ps")
            for i, (off, sz) in enumerate(SP3_TILES):
                nc.tensor.matmul(ps[:], x3T[i][:, mt * 128:(mt + 1) * 128],
                                 sm[i][0:sz, :], start=(i == 0), stop=(i == 1))
            t = T(pp, [128, PH2], BF16, f"featT{8 + mt}")
            nc.vector.tensor_copy(t[:], ps[:])
            featT[8 + mt] = t

        ht_block(ps_ht, [0, 1, 2, 3], [8, 9, 10, 11], False, True)
        htT = [None] * 8
        for mt in range(4):
            t = T(pp, [128, PH2], BF16, f"htT{mt}")
            nc.scalar.activation(
                t[:], ps_ht[mt][:], ACT.Tanh,
                bias=b32_t[:, B32_L["bht"] + mt:B32_L["bht"] + mt + 1])
            htT[mt] = t
        ps_ht2 = [T(pps, [128, PH2], FP32, "psht", bufs=4) for _ in range(4)]
        ht_block(ps_ht2, [4, 5, 6, 7], list(range(16)), True, True)
        for mt in range(4, 8):
            t = T(pp, [128, PH2], BF16, f"htT{mt}")
            nc.scalar.activation(
                t[:], ps_ht2[mt - 4][:], ACT.Tanh,
                bias=b32_t[:, B32_L["bht"] + mt:B32_L["bht"] + mt + 1])
            htT[mt] = t

        ps_out = T(pps, [97, PH2], FP32, "ps")
        for kt in range(8):
            nc.tensor.matmul(ps_out[:], wbil_t[:, kt * 97:(kt + 1) * 97],
                             htT[kt][:], start=(kt == 0), stop=(kt == 7))
        out_sb = T(pp, [97, PH2], FP32, "out")
        nc.vector.tensor_scalar_add(
            out_sb[:], ps_out[:],
            b32_t[0:97, B32_L["bbil"]:B32_L["bbil"] + 1])
        # split across two HWDGE rings to halve the packet-rate-bound tail
        dma(outt[:, 0:PH2 // 2], out_sb[:, 0:PH2 // 2])
        nc.scalar.dma_start(outt[:, PH2 // 2:PH2], out_sb[:, PH2 // 2:PH2])

    nc.compile()
    return nc


_PROG = None


def _get_prog():
    global _PROG
    if _PROG is None:
        _PROG = build_program()
    return _PROG


def _padtiles(arr, tile_rows=128):
    """[R, W] -> [128, ceil(R/128)*W] column-blocked, rows zero-padded."""
    r, w = arr.shape
    nt = (r + tile_rows - 1) // tile_rows
    out = np.zeros((tile_rows, nt * w), arr.dtype)
    for i in range(nt):
        blk = arr[i * tile_rows:(i + 1) * tile_rows]
        out[:blk.shape[0], i * w:i * w + w] = blk
    return out


def _shared_inputs(inputs):
    f32 = np.float32
    bf = ml_dtypes.bfloat16
    sh = {}
    sh["identp"] = np.eye(128, dtype=bf)
    # blob32 + single-row blob
    b32 = np.zeros((128, B32_W), f32)
    b32[:, B32_L["ident"]:B32_L["ident"] + 128] = np.eye(128, dtype=f32)
    b32[:, B32_L["ones"]] = 1.0
    brow = np.zeros((1, BROW_W), f32)
    brow[0, BROW_L["onesrow"]:BROW_L["onesrow"] + 128] = 1.0
    brow[0, BROW_L["btrans"]:BROW_L["btrans"] + 512] = np.asarray(
        inputs["b_trans"], f32)
    brow[0, BROW_L["brgcn"]:BROW_L["brgcn"] + 512] = np.asarray(
        inputs["b_rgcn"], f32)
    sh["brow"] = brow
    b32[:, B32_L["b1"]:B32_L["b1"] + 2] = np.asarray(
        inputs["conv1_b"], f32).reshape(2, 128).T
    b32[:, B32_L["b2"]:B32_L["b2"] + 2] = np.asarray(
        inputs["conv2_b"], f32).reshape(2, 128).T
    b32[:, B32_L["b3"]:B32_L["b3"] + 4] = np.asarray(
        inputs["conv3_b"], f32).reshape(4, 128).T
    b32[:, B32_L["bht"]:B32_L["bht"] + 8] = np.asarray(
        inputs["ht_b"], f32).reshape(8, 128).T
    b32[0:97, B32_L["bbil"]] = np.asarray(inputs["bil_b"], f32)
    sh["b32"] = b32
    # wrel: 20 full blocks (r, kt<4) of [128, 512] + 5 tail blocks [20, 512]
    wcat = np.concatenate(
        [np.asarray(inputs["W_rel"], f32).reshape(4 * NF, EMB),
         np.asarray(inputs["W_self"], f32)], axis=0)
    wr = np.zeros((128, 20 * 512), f32)
    wt = np.zeros((20, 5 * 512), f32)
    for r in range(5):
        for i in range(4):
            blk = wcat[r * NF + i * 128:r * NF + (i + 1) * 128]
            bidx = r * 4 + i
            col = (bidx // 13) * WCH + (bidx % 13) * 512
            wr[:, col:col + 512] = blk
        wt[:, r * 512:(r + 1) * 512] = wcat[r * NF + 512:r * NF + NF]
    sh["wrel"] = wr.astype(bf)
    sh["wtail"] = wt.astype(bf)
    # conv weights, two spatial variants
    def convpack(w, nkt, oc, nchunk):
        # w [oc, ic, 5, 5] -> [128, nchunk*WCH]; within a chunk, col tap*oc + o
        t = np.ascontiguousarray(np.asarray(w, f32).transpose(1, 2, 3, 0))
        t = t.reshape(nkt, 128, 25, oc).transpose(1, 0, 2, 3)  # [128,kt,tap,oc]
        out = np.zeros((128, nchunk * WCH), f32)
        taps_per = 25 // (nchunk // nkt) if nchunk != nkt else 25
        if nchunk == nkt:           # whole kt fits one chunk (25*oc <= WCH)
            for kt in range(nkt):
                out[:, kt * WCH:kt * WCH + 25 * oc] = t[:, kt].reshape(128, -1)
        else:                       # kt split into 2 chunks: taps 0-12 / 13-24
            for kt in range(nkt):
                for half, (t0, t1) in enumerate([(0, 13), (13, 25)]):
                    ch = kt * 2 + half
                    blk = t[:, kt, t0:t1].reshape(128, -1)
                    out[:, ch * WCH:ch * WCH + blk.shape[1]] = blk
        return np.ascontiguousarray(out).astype(bf)
    for hh in range(2):
        w1 = np.asarray(inputs["conv1_w"], f32)
        w2 = np.asarray(inputs["conv2_w"], f32)
        w3 = np.asarray(inputs["conv3_w"], f32)
        if hh == 1:
            w1 = w1[:, :, ::-1, ::-1]
            w2 = w2[:, :, ::-1, ::-1]
            w3 = w3[:, :, ::-1, ::-1]
        sh[f"w1_{hh}"] = convpack(w1, 4, IC, 4)
        sh[f"w2_{hh}"] = convpack(w2, 2, IC, 2)
        sh[f"w3_{hh}"] = convpack(w3, 2, EMB, 4)
    sh["wht"] = np.ascontiguousarray(
        np.asarray(inputs["ht_W"], f32).reshape(16, 128, 1024)
        .transpose(1, 0, 2).reshape(128, 16 * 1024)).astype(bf)
    sh["wbil"] = np.ascontiguousarray(
        np.asarray(inputs["bil_W"], f32).reshape(8, 128, 97)
        .transpose(1, 0, 2).reshape(128, 8 * 97)).astype(bf)
    # structural constants
    sh["_g3"] = np.kron(np.eye(E, dtype=f32), np.ones((M, 1), f32))
    sh["_gmat"] = np.kron(np.eye(E, dtype=f32),
                          np.ones((M * NH, 1), f32) / (M * NH))
    sh["_gspan"] = np.kron(np.eye(L, dtype=f32), np.ones((LS, 1), f32))
    sh["_wtrans"] = np.asarray(inputs["W_trans"], f32)
    sh["_type_embed"] = np.asarray(inputs["type_embed"], f32)
    return sh


def _core_inputs(inputs, shared, b, hh):
    f32 = np.float32
    bf = ml_dtypes.bfloat16
    if hh == 0:
        perm_e = np.arange(E)
    else:
        perm_e = np.arange(E)[::-1]
    perm_m = (perm_e[:, None] * M + np.arange(M)).reshape(-1)
    p_nodes = np.concatenate([perm_e, E + perm_m, np.arange(E + E * M, NN)])
    X = np.asarray(inputs["sequence_output"][b], f32)
    att = np.asarray(inputs["attention"][b], f32)
    adj = np.asarray(inputs["adjacency"][b], f32)[:, p_nodes][:, :, p_nodes]
    mf = np.asarray(inputs["mention_idx"][b]).astype(np.int64)[perm_e].reshape(-1)
    ls = np.asarray(inputs["link_start"][b]).astype(np.int64)
    ntypes = np.asarray(inputs["node_types"][b]).astype(np.int64)[p_nodes]
    hts = np.asarray(inputs["hts"][b]).astype(np.int64)

    # C0a / C0b
    c0a = np.zeros((128, C0A_W), bf)
    c0b = np.zeros((128, C0B_W), bf)
    adjrel = np.zeros((NN, 110), f32)
    for r in range(4):
        adjrel[:, r * 22:(r + 1) * 22] = adj[r].T[:, :E]
    adjrel[:, 88:110] = np.eye(NN, dtype=f32)[:, :E]
    typ_all = shared["_type_embed"][ntypes]
    goff = 0
    for gi, g in enumerate([22, 66, 30]):
        c0a[0:g, C0A_L[f"adjrel{gi}"]:C0A_L[f"adjrel{gi}"] + 110] = \
            adjrel[goff:goff + g].astype(bf)
        c0a[0:g, C0A_L[f"typ{gi}"]:C0A_L[f"typ{gi}"] + 20] = \
            typ_all[goff:goff + g].astype(bf)
        goff += g
    c0a[0:E * M, C0A_L["g3"]:C0A_L["g3"] + 22] = shared["_g3"].astype(bf)
    c0a[:, C0A_L["ones16"]] = 1.0
    c0a[:, C0A_L["gmat"]:C0A_L["gmat"] + GM_TILES * 22] = _padtiles(
        shared["_gmat"].astype(bf))
    c0a[:, C0A_L["gspan"]:C0A_L["gspan"] + SPAN_TILES * 30] = _padtiles(
        shared["_gspan"].astype(bf))
    pos = ls[:, None] + np.arange(LS)
    attl = np.empty((L * LS, NH * LS), f32)
    for l in range(L):
        blk = att[:, pos[l]][:, :, pos[l]]
        attl[l * LS:(l + 1) * LS] = blk.transpose(2, 0, 1).reshape(LS, NH * LS)
    c0a[:, C0A_L["attl"]:C0A_L["attl"] + SPAN_TILES * 192] = _padtiles(
        attl.astype(bf))

    def _fp8(x, scale):
        return np.clip(np.asarray(x, f32) * scale, -240.0, 240.0).astype(
            ml_dtypes.float8_e4m3)
    c0b[:, C0B_L["xg"]:C0B_L["xg"] + 6 * 66] = _padtiles(
        np.ascontiguousarray(X[mf].T).astype(bf))
    c0b[:, C0B_L["wtrans"]:C0B_L["wtrans"] + 6 * 512] = _padtiles(
        shared["_wtrans"].astype(bf))
    # C1 xspan, C2 attm (fp8, scaled), C3 xn (fp8, scaled)
    c1 = _padtiles(X[pos.reshape(-1)].astype(bf))
    attm = np.ascontiguousarray(
        att[:, mf].transpose(1, 0, 2).reshape(E * M * NH, C))
    c2 = _padtiles(_fp8(attm, S_ATT))
    c3 = _padtiles(_fp8(X, S_X))
    # C4 selectors (local band: h' in 0..10)
    inv = np.empty(E, np.int64)
    inv[perm_e] = np.arange(E)
    hl, tl = inv[hts[:, 0]], inv[hts[:, 1]]
    sel = np.where(hl <= 10)[0]
    assert len(sel) <= PH2, f"band overflow: {len(sel)} > {PH2}"
    c4 = np.zeros((128, C4_W), bf)
    shm = np.zeros((E, PH2), f32)
    shm[hl[sel], np.arange(len(sel))] = 1.0
    stm = np.zeros((E, PH2), f32)
    stm[tl[sel], np.arange(len(sel))] = 1.0
    smm = np.zeros((SP3, PH2), f32)
    smm[hl[sel] * S + tl[sel], np.arange(len(sel))] = 1.0
    c4[0:E, C4_L["sh"]:C4_L["sh"] + PH2] = shm.astype(bf)
    c4[0:E, C4_L["st"]:C4_L["st"] + PH2] = stm.astype(bf)
    c4[:, C4_L["sm"]:C4_L["sm"] + 2 * PH2] = _padtiles(smm.astype(bf))

    m = {
        "identp": shared["identp"], "b32": shared["b32"],
        "brow": shared["brow"],
        "wrel": shared["wrel"], "wtail": shared["wtail"],
        "wht": shared["wht"], "wbil": shared["wbil"],
        "w1": shared[f"w1_{hh}"], "w2": shared[f"w2_{hh}"],
        "w3": shared[f"w3_{hh}"],
        "c0a": c0a, "c0b": c0b, "c1": c1, "c2": c2, "c3": c3, "c4": c4,
    }
    return m, sel


def kernel(**inputs):
    nc = _get_prog()
    shared = _shared_inputs(inputs)
    in_maps, sels = [], []
    for b in range(B):
        for hh in range(2):
            m, sel = _core_inputs(inputs, shared, b, hh)
            in_maps.append(m)
            sels.append(sel)
    res = run_bass_kernel_spmd(nc, in_maps, core_ids=list(range(8)))
    out = np.empty((B, P, 97), np.float32)
    for b in range(B):
        for hh in range(2):
            ci = 2 * b + hh
            sel = sels[ci]
            r = np.asarray(res.results[ci]["outt"], np.float32)
            out[b, sel, :] = r[:, :len(sel)].T
    return out
